# revision 1
# baseline (speedup 1.0000x reference)
"""GAT spatio-temporal model Trainium2 kernel (v3).

Sharding: data-parallel over batch B=8 -> 8 NeuronCores (1 graph each).
Layout: feature-on-partition ("T" tensors are [F, N]); attention computed
in transposed [m, n] layout so softmax denominators come from ones-matmul
column sums and AV products hit PE directly.

v3: per-layer two-stage structure -- stage A computes projections,
attention rows and z replications for ALL heads (deep pools so the
engines run dense, back-to-back work), stage B runs the latency-heavy
softmax/AV bodies overlapped across heads. bf16 operands on the N^2
paths, leaky-relu fused on ScalarE via Prelu+per-partition bias.

Shapes (hardcoded): B=8, N=512, Din=64, H=8, F=128, L=2.
"""
import os
import numpy as np
from contextlib import ExitStack

import concourse.bass as bass
import concourse.tile as tile
from concourse import bacc, mybir
from concourse.bass_utils import run_bass_kernel_spmd
from concourse.masks import make_identity

F32 = mybir.dt.float32
BF16 = mybir.dt.bfloat16
AF = mybir.ActivationFunctionType
OP = mybir.AluOpType

B, N, DIN, H, F, L = 8, 512, 64, 8, 128, 2
NCHUNK = N // 128  # 4
ALPHA = 0.2
LN_EPS = 1e-5
ACT_CHUNKS = int(os.environ.get("K_ACT_CHUNKS", "4"))
USE_BCAST = os.environ.get("K_BCAST", "1") == "1"

_CACHE = {}


def _bcast_row(ap_row):
    return bass.AP(tensor=ap_row.tensor, offset=ap_row.offset, ap=[[0, 128], [1, N]])


def build_nc():
    nc = bacc.Bacc("TRN2", target_bir_lowering=False, debug=False)

    x_d = nc.dram_tensor("x", [N, DIN], F32, kind="ExternalInput").ap()
    adj_d = nc.dram_tensor("adj", [N, N], mybir.dt.int32, kind="ExternalInput").ap()
    Wp_d = nc.dram_tensor("Wp", [DIN, F], F32, kind="ExternalInput").ap()
    bp_d = nc.dram_tensor("bp", [F], F32, kind="ExternalInput").ap()
    Wh_d = nc.dram_tensor("W_heads", [L, H, F, F], F32, kind="ExternalInput").ap()
    ah_d = nc.dram_tensor("a_heads", [L, H, 2 * F], F32, kind="ExternalInput").ap()
    Wo_d = nc.dram_tensor("W_out", [L, H * F, F], F32, kind="ExternalInput").ap()
    ao_d = nc.dram_tensor("a_out", [L, 2 * F], F32, kind="ExternalInput").ap()
    g_d = nc.dram_tensor("ln_g", [L, F], F32, kind="ExternalInput").ap()
    b_d = nc.dram_tensor("ln_b", [L, F], F32, kind="ExternalInput").ap()
    out_d = nc.dram_tensor("out", [N, F], F32, kind="ExternalOutput").ap()
    scr_d = [nc.dram_tensor(f"scratch{i}", [1, N], F32, kind="ExternalOutput").ap()
             for i in range(22)]

    with tile.TileContext(nc) as tc, ExitStack() as ctx:
        const = ctx.enter_context(tc.tile_pool(name="const", bufs=1))
        sproj = ctx.enter_context(tc.tile_pool(name="sproj", bufs=9))
        sprt = ctx.enter_context(tc.tile_pool(name="sprt", bufs=3))
        sbig = ctx.enter_context(tc.tile_pool(name="sbig", bufs=3))
        srow = ctx.enter_context(tc.tile_pool(name="srow", bufs=4))
        sexp_e = ctx.enter_context(tc.tile_pool(name="sexp_e", bufs=2))
        sexp_p = ctx.enter_context(tc.tile_pool(name="sexp_p", bufs=4))
        smulti = ctx.enter_context(tc.tile_pool(name="smulti", bufs=9))
        shd = ctx.enter_context(tc.tile_pool(name="shd", bufs=2))
        smask = ctx.enter_context(tc.tile_pool(name="smask", bufs=4))
        pou = ctx.enter_context(tc.tile_pool(name="pou", bufs=3, space="PSUM"))
        pmisc = ctx.enter_context(tc.tile_pool(name="pmisc", bufs=3, space="PSUM"))
        prow = ctx.enter_context(tc.tile_pool(name="prow", bufs=2, space="PSUM"))

        # ---------------- constants ----------------
        ones_row = const.tile([1, N], F32)
        nc.vector.memset(ones_row, 1.0)
        ones_col_bf = const.tile([128, 1], BF16)
        nc.vector.memset(ones_col_bf, 1.0)
        ones_col = const.tile([128, 1], F32)
        nc.vector.memset(ones_col, 1.0)
        ident = const.tile([128, 128], F32)
        make_identity(nc, ident)
        eps1 = const.tile([1, 1], F32)
        nc.vector.memset(eps1, LN_EPS)

        Wp_sb = const.tile([DIN, F], F32)
        nc.sync.dma_start(Wp_sb, Wp_d)
        bp_col = const.tile([F, 1], F32)
        nc.sync.dma_start(bp_col, bp_d.rearrange("(f one) -> f one", one=1))
        x_chunks = []
        for c in range(NCHUNK):
            xc = shd.tile([128, DIN], F32, tag="xchunk")
            nc.sync.dma_start(xc, x_d[bass.ts(c, 128), :])
            x_chunks.append(xc)

        # Per-layer weight loads on separate DMA queues: layer 0 lands
        # first (sync HWDGE) so stage A starts immediately; layer 1 and
        # W_out trickle in behind on gpsimd/scalar queues.
        Wh_all = [const.tile([F, H, F], F32, name=f"WhA{l}") for l in range(L)]
        Wh_ball = [const.tile([F, H, F], BF16, name=f"WhB{l}") for l in range(L)]
        nc.sync.dma_start(Wh_all[0], Wh_d[0].rearrange("h i o -> i h o"))
        nc.gpsimd.dma_start(Wh_all[1], Wh_d[1].rearrange("h i o -> i h o"))
        for l in range(L):
            nc.vector.tensor_copy(Wh_ball[l], Wh_all[l])
        Wh_sb = [[Wh_all[l][:, h, :] for h in range(H)] for l in range(L)]
        Wh_bf = [[Wh_ball[l][:, h, :] for h in range(H)] for l in range(L)]
        ah_all = const.tile([F, L * H, 2], F32)
        nc.sync.dma_start(ah_all, ah_d.rearrange("l h (t f) -> f (l h) t", t=2))
        ah_sb = [[ah_all[:, l * H + h, :] for h in range(H)] for l in range(L)]
        wo_f = [const.tile([128, H, F], F32, name=f"WoF{l}") for l in range(L)]
        Wo_ball = [const.tile([128, H, F], BF16, name=f"WoB{l}") for l in range(L)]
        for l in range(L):
            nc.gpsimd.dma_start(wo_f[l], Wo_d[l].rearrange("(c p) f -> p c f", p=128))
            nc.vector.tensor_copy(Wo_ball[l], wo_f[l])
        Wo_bf = [Wo_ball[l] for l in range(L)]
        ao_all = const.tile([F, L, 2], F32)
        nc.sync.dma_start(ao_all, ao_d.rearrange("l (t f) -> f l t", t=2))
        ao_sb = [ao_all[:, l, :] for l in range(L)]
        g_row = [const.tile([1, F], F32, name=f"grow_{l}") for l in range(L)]
        b_row = [const.tile([1, F], F32, name=f"brow_{l}") for l in range(L)]
        g_col = [const.tile([F, 1], F32, name=f"gcol_{l}") for l in range(L)]
        for l in range(L):
            nc.sync.dma_start(g_row[l], g_d[l].rearrange("(one f) -> one f", one=1))
            nc.sync.dma_start(b_row[l], b_d[l].rearrange("(one f) -> one f", one=1))
            nc.sync.dma_start(g_col[l], g_d[l].rearrange("(f one) -> f one", one=1))

        # ---------------- x -> xT, input projection (chunk-wise) ----------------
        xT = const.tile([DIN, N], F32)
        ph = pmisc.tile([128, N], F32, tag="pbig")
        hT = sbig.tile([128, N], F32, tag="hT")
        hT_bf = sbig.tile([128, N], BF16, tag="hTb")
        for c in range(NCHUNK):
            pt = pmisc.tile([DIN, 128], F32, tag="pbig")
            nc.tensor.transpose(pt, x_chunks[c], ident)
            nc.scalar.activation(xT[:, bass.ts(c, 128)], pt, AF.Copy)
            nc.tensor.matmul(ph[:, bass.ts(c, 128)], Wp_sb, xT[:, bass.ts(c, 128)],
                             start=True, stop=True)
            nc.scalar.activation(hT[:, bass.ts(c, 128)], ph[:, bass.ts(c, 128)],
                                 AF.Relu, bias=bp_col)
            nc.vector.tensor_copy(hT_bf[:, bass.ts(c, 128)], hT[:, bass.ts(c, 128)])

        # ---------------- adj -> maskT (bf16, transposed) ----------------
        adj_f = []
        for r in range(NCHUNK):
            ai = shd.tile([128, N], mybir.dt.int32, tag="adji")
            eng = nc.scalar if r % 2 == 0 else nc.sync
            eng.dma_start(ai, adj_d[bass.ts(r, 128), :])
            af = smask.tile([128, N], F32, tag="adjf")
            nc.vector.tensor_copy(af, ai)
            adj_f.append(af)
        maskT = [const.tile([128, N], BF16, name=f"maskT{c}") for c in range(NCHUNK)]
        # One small psum per (c,r) block, evacuated immediately: slots
        # recycle as each adj chunk lands instead of holding a bank while
        # the full 1MB adjacency streams in.
        for r in range(NCHUNK):
            for c in range(NCHUNK):
                pm = pmisc.tile([128, 128], F32, tag="pbig")
                nc.tensor.transpose(pm, adj_f[r][:, bass.ts(c, 128)], ident)
                nc.scalar.activation(maskT[c][:, bass.ts(r, 128)], pm, AF.Copy)

        # ---------------- stage helpers ----------------
        def stage_a(projT, a_cols, hid, use_bcast=True):
            """Rows + z replication for one attention. projT fp32 [F,N].
            Returns (z_sb, s2cols)."""
            s12p = prow.tile([2, N], F32, tag="prow")
            nc.tensor.matmul(s12p, a_cols, projT, start=True, stop=True)
            s12row = srow.tile([2, N], F32, tag="rowA")
            nc.vector.tensor_copy(s12row, s12p)
            z_sb = sproj.tile([128, N], F32, tag="z_sb")
            s2cols = sproj.tile([128, 4], F32, tag="s2cols")
            if use_bcast:
                nc.sync.dma_start(scr_d[hid], s12row[0:1, :])
                nc.sync.dma_start(z_sb, _bcast_row(scr_d[hid][0, :]))
            else:
                prz = pmisc.tile([128, N], F32, tag="pbig")
                nc.tensor.matmul(prz, ones_row[:, 0:128], s12row[0:1, :],
                                 start=True, stop=True)
                nc.scalar.activation(z_sb, prz, AF.Copy)
            nc.sync.dma_start(scr_d[10 + hid], s12row[1:2, :])
            s2scr = scr_d[10 + hid][0, :]
            nc.sync.dma_start(s2cols, bass.AP(tensor=s2scr.tensor, offset=s2scr.offset,
                                              ap=[[1, 128], [128, 4]]))
            return z_sb, s2cols

        def stage_b(z_sb, s2cols, projN_bf, hid, act_chunks=None, use_bcast=True):
            """Softmax + AV for one attention. Returns (pou_t, rep_sb)."""
            if act_chunks is None:
                act_chunks = ACT_CHUNKS
            e_all = sexp_e.tile([128, NCHUNK, N], F32, tag="e_all")
            p_all = sexp_p.tile([128, NCHUNK, N], BF16, tag="p_all")
            for c in range(NCHUNK):
                if c < act_chunks:
                    nc.scalar.activation(e_all[:, c, :], z_sb, AF.Prelu,
                                         bias=s2cols[:, c:c + 1], alpha=ALPHA)
                else:
                    u = shd.tile([128, N], F32, tag="lrelu_u")
                    nc.vector.tensor_scalar_add(u, z_sb, s2cols[:, c:c + 1])
                    t = shd.tile([128, N], F32, tag="lrelu_t")
                    nc.vector.tensor_scalar_mul(t, u, ALPHA)
                    nc.vector.tensor_tensor(e_all[:, c, :], u, t, OP.max)
            nc.scalar.activation(p_all, e_all, AF.Exp)
            for c in range(NCHUNK):
                nc.vector.tensor_tensor(p_all[:, c, :], p_all[:, c, :], maskT[c],
                                        OP.mult)
            pden = prow.tile([1, N], F32, tag="prow")
            pou_t = pou.tile([128, N], F32, tag="oU")
            for c in range(NCHUNK):
                nc.tensor.matmul(pden, ones_col_bf, p_all[:, c, :],
                                 start=(c == 0), stop=(c == NCHUNK - 1))
            for c in range(NCHUNK):
                nc.tensor.matmul(pou_t, projN_bf[:, bass.ts(c, 128)], p_all[:, c, :],
                                 start=(c == 0), stop=(c == NCHUNK - 1))
            r_sb = srow.tile([1, N], F32, tag="rowR")
            nc.vector.reciprocal_approx_fast(r_sb, pden)
            rep_sb = sbig.tile([128, N], F32, tag="rep")
            if use_bcast:
                nc.sync.dma_start(scr_d[8 + hid % 2], r_sb)
                nc.sync.dma_start(rep_sb, _bcast_row(scr_d[8 + hid % 2][0, :]))
            else:
                prr = pmisc.tile([128, N], F32, tag="pbig")
                nc.tensor.matmul(prr, ones_row[:, 0:128], r_sb, start=True, stop=True)
                nc.scalar.activation(rep_sb, prr, AF.Copy)
            return pou_t, rep_sb

        # ---------------- layers ----------------
        for l in range(L):
            residT = hT
            # ---- stage A: projections + rows for all heads ----
            hprojN = []
            zs = []
            for h in range(H):
                pT = pmisc.tile([128, N], F32, tag="pbig")
                nc.tensor.matmul(pT, Wh_sb[l][h], hT, start=True, stop=True)
                hprojT = sprt.tile([128, N], F32, tag="hprojT")
                nc.scalar.activation(hprojT, pT, AF.Copy)
                pN = pmisc.tile([128, N], F32, tag="pbig")
                for c in range(NCHUNK):
                    nc.tensor.matmul(pN[:, bass.ts(c, 128)], hT_bf[:, bass.ts(c, 128)],
                                     Wh_bf[l][h], start=True, stop=True)
                pn_bf = sproj.tile([128, N], BF16, tag="hprojN")
                nc.vector.tensor_copy(pn_bf, pN)
                hprojN.append(pn_bf)
                zs.append(stage_a(hprojT, ah_sb[l][h], h))
            # ---- stage B: attention bodies ----
            multiT = []
            for h in range(H):
                z_sb, s2cols = zs[h]
                pou_t, rep_sb = stage_b(z_sb, s2cols, hprojN[h], h)
                outT = sbig.tile([128, N], BF16, tag="outT")
                nc.vector.tensor_tensor(outT, pou_t, rep_sb, OP.mult)
                ex = shd.tile([128, N], BF16, tag="elu_ex")
                nc.scalar.activation(ex, outT, AF.Exp)
                nc.vector.tensor_scalar(ex, ex, 1.0, -1.0, OP.min, OP.add)
                mh = smulti.tile([128, N], BF16, tag="multi")
                nc.vector.tensor_tensor(mh, outT, ex, OP.max)
                multiT.append(mh)

            ph2 = pou.tile([128, N], F32, tag="oU")
            for h in range(H):
                nc.tensor.matmul(ph2, Wo_bf[l][:, h, :], multiT[h],
                                 start=(h == 0), stop=(h == H - 1))
            h2T = sbig.tile([128, N], F32, tag="h2T")
            nc.scalar.activation(h2T, ph2, AF.Copy)
            h2N_bf = sbig.tile([128, N], BF16, tag="h2N")
            pn2 = pmisc.tile([128, N], F32, tag="pbig")
            for c in range(NCHUNK):
                nc.tensor.transpose(pn2[:, bass.ts(c, 128)], h2T[:, bass.ts(c, 128)],
                                    ident)
            nc.vector.tensor_copy(h2N_bf, pn2)
            z_sb, s2cols = stage_a(h2T, ao_sb[l], 8, use_bcast=False)
            pou_t, rep_sb = stage_b(z_sb, s2cols, h2N_bf, l, act_chunks=3,
                                    use_bcast=False)
            outsT = sbig.tile([128, N], F32, tag="outT")
            nc.vector.tensor_tensor(outsT, pou_t, rep_sb, OP.mult)

            # ---- residual + LN over partition dim ----
            xs = sbig.tile([128, N], F32, tag="xs")
            nc.vector.tensor_tensor(xs, outsT, residT, OP.add)
            xsq = sbig.tile([128, N], F32, tag="xsq")
            nc.vector.tensor_tensor(xsq, xs, xs, OP.mult)
            pmu = prow.tile([1, N], F32, tag="prow")
            nc.tensor.matmul(pmu, ones_col, xs, start=True, stop=True)
            psq = prow.tile([1, N], F32, tag="prow")
            nc.tensor.matmul(psq, ones_col, xsq, start=True, stop=True)
            mu = srow.tile([1, N], F32, tag="rowL")
            nc.vector.tensor_scalar_mul(mu, pmu, 1.0 / F)
            msq = srow.tile([1, N], F32, tag="rowL")
            nc.vector.tensor_scalar_mul(msq, psq, 1.0 / F)
            mu2 = srow.tile([1, N], F32, tag="rowL")
            nc.vector.tensor_tensor(mu2, mu, mu, OP.mult)
            var = srow.tile([1, N], F32, tag="rowL")
            nc.vector.tensor_tensor(var, msq, mu2, OP.subtract)
            lnv = srow.tile([1, N], F32, tag="rowL")
            nc.scalar.activation(lnv, var, AF.Ln, bias=eps1)
            rstd = srow.tile([1, N], F32, tag="rowL")
            nc.scalar.activation(rstd, lnv, AF.Exp, scale=-0.5)
            mr = srow.tile([1, N], F32, tag="rowL")
            nc.vector.tensor_tensor(mr, mu, rstd, OP.mult)
            r2 = srow.tile([1, N], F32, tag="rowL")
            nc.vector.tensor_scalar_mul(r2, mr, -1.0)
            paff = pmisc.tile([128, N], F32, tag="pbig")
            nc.tensor.matmul(paff, g_row[l], r2, start=True, stop=False)
            nc.tensor.matmul(paff, b_row[l], ones_row, start=False, stop=True)
            prs = pmisc.tile([128, N], F32, tag="pbig")
            nc.tensor.matmul(prs, ones_row[:, 0:128], rstd, start=True, stop=True)
            rep_rstd = sbig.tile([128, N], F32, tag="rep")
            nc.scalar.activation(rep_rstd, prs, AF.Copy)
            y = sbig.tile([128, N], F32, tag="y")
            nc.vector.tensor_tensor(y, xs, rep_rstd, OP.mult)
            nc.vector.tensor_scalar_mul(y, y, g_col[l])
            hT_new = sbig.tile([128, N], F32, tag="hT")
            nc.vector.tensor_tensor(hT_new, y, paff, OP.add)
            if l < L - 1:
                nc.vector.tensor_scalar_max(hT_new, hT_new, 0.0)
            hT = hT_new
            if l < L - 1:
                hT_bf = sbig.tile([128, N], BF16, tag="hTb")
                nc.vector.tensor_copy(hT_bf, hT)

        # ---------------- output: transpose back ----------------
        for c in range(NCHUNK):
            po = pmisc.tile([128, 128], F32, tag="pbig")
            nc.tensor.transpose(po, hT[:, bass.ts(c, 128)], ident)
            osb = shd.tile([128, 128], F32, tag="osb")
            nc.scalar.activation(osb, po, AF.Copy)
            nc.sync.dma_start(out_d[bass.ts(c, 128), :], osb)

    nc.compile()
    return nc


def _get_nc():
    if "nc" not in _CACHE:
        _CACHE["nc"] = build_nc()
    return _CACHE["nc"]


def kernel(**inputs) -> np.ndarray:
    nc = _get_nc()
    shared = {k: np.ascontiguousarray(np.asarray(inputs[k], dtype=np.float32))
              for k in ("Wp", "bp", "W_heads", "a_heads", "W_out", "a_out",
                        "ln_g", "ln_b")}
    x = np.asarray(inputs["x"], dtype=np.float32)
    adj = np.asarray(inputs["adj"], dtype=np.int32)
    in_maps = [dict(x=np.ascontiguousarray(x[b]),
                    adj=np.ascontiguousarray(adj[b]), **shared)
               for b in range(B)]
    res = run_bass_kernel_spmd(nc, in_maps, core_ids=list(range(B)))
    return np.stack([res.results[b]["out"] for b in range(B)])


if __name__ == "__main__":
    rng = np.random.default_rng(0)
    inputs = dict(
        x=rng.normal(size=(B, N, DIN)).astype(np.float32),
        adj=rng.integers(0, 2, size=(B, N, N)).astype(np.int32),
        Wp=(rng.normal(size=(DIN, F)) * 0.12).astype(np.float32),
        bp=np.zeros(F, dtype=np.float32),
        W_heads=(rng.normal(size=(L, H, F, F)) * 0.08).astype(np.float32),
        a_heads=(rng.normal(size=(L, H, 2 * F)) * 0.08).astype(np.float32),
        W_out=(rng.normal(size=(L, H * F, F)) * 0.03).astype(np.float32),
        a_out=(rng.normal(size=(L, 2 * F)) * 0.08).astype(np.float32),
        ln_g=np.ones((L, F), dtype=np.float32),
        ln_b=np.zeros((L, F), dtype=np.float32),
    )
    out = kernel(**inputs)
    print("out", out.shape, out.dtype, np.abs(out).max())



# revision 10
# speedup vs baseline: 1.2087x; 1.2087x over previous
"""GAT spatio-temporal model Trainium2 kernel (v5).

Sharding: data-parallel over batch B=8 -> 8 NeuronCores (1 graph each).
Layout: feature-on-partition ("T" tensors are [F, N]); attention computed
in transposed [m, n] layout so softmax denominators come from ones-matmul
column sums and AV products hit PE directly.

v5 vs v3:
- z (s1 broadcast) comes from one bf16 matmul with a prebuilt rank-1
  stationary M_h = a1Wh^T broadcast along free dim: z = M_h^T @ hT.
  No per-head fp32 projection, no s12 row extraction, no DRAM round
  trips (v3 burned 80us of sync-queue time on those).
- s2 bias columns come from 4 tiny [128,16] chunk matmuls (hT^T A).
- r / rstd broadcasts are f32r ones-matmuls (1 cyc/row at 512 cols).
- All big PE moving operands are bf16.
- Mask multiply is one [128,4*512] DVE op per head (bf16 2x mode).
- LN rows fused via scalar_tensor_tensor; rsqrt via Sqrt + fast recip.

Shapes (hardcoded): B=8, N=512, Din=64, H=8, F=128, L=2.
"""
import os
import numpy as np
from contextlib import ExitStack

import concourse.bass as bass
import concourse.tile as tile
from concourse import bacc, mybir
from concourse.bass_utils import run_bass_kernel_spmd
from concourse.masks import make_identity

F32 = mybir.dt.float32
F32R = mybir.dt.float32r
BF16 = mybir.dt.bfloat16
AF = mybir.ActivationFunctionType
OP = mybir.AluOpType

B, N, DIN, H, F, L = 8, 512, 64, 8, 128, 2
NCHUNK = N // 128  # 4
ALPHA = 0.2
LN_EPS = 1e-5

_CACHE = {}


def build_nc():
    nc = bacc.Bacc("TRN2", target_bir_lowering=False, debug=False)

    x_d = nc.dram_tensor("x", [N, DIN], F32, kind="ExternalInput").ap()
    adj_d = nc.dram_tensor("adj", [N, N], mybir.dt.int32, kind="ExternalInput").ap()
    Wp_d = nc.dram_tensor("Wp", [DIN, F], F32, kind="ExternalInput").ap()
    bp_d = nc.dram_tensor("bp", [F], F32, kind="ExternalInput").ap()
    Wh_d = nc.dram_tensor("W_heads", [L, H, F, F], F32, kind="ExternalInput").ap()
    ah_d = nc.dram_tensor("a_heads", [L, H, 2 * F], F32, kind="ExternalInput").ap()
    Wo_d = nc.dram_tensor("W_out", [L, H * F, F], F32, kind="ExternalInput").ap()
    ao_d = nc.dram_tensor("a_out", [L, 2 * F], F32, kind="ExternalInput").ap()
    g_d = nc.dram_tensor("ln_g", [L, F], F32, kind="ExternalInput").ap()
    b_d = nc.dram_tensor("ln_b", [L, F], F32, kind="ExternalInput").ap()
    out_d = nc.dram_tensor("out", [N, F], F32, kind="ExternalOutput").ap()

    with tile.TileContext(nc) as tc, ExitStack() as ctx:
        const = ctx.enter_context(tc.tile_pool(name="const", bufs=1))
        stmp = ctx.enter_context(tc.tile_pool(name="stmp", bufs=2))
        sproj = ctx.enter_context(tc.tile_pool(name="sproj", bufs=10))
        smulti = ctx.enter_context(tc.tile_pool(name="smulti", bufs=9))
        sbig = ctx.enter_context(tc.tile_pool(name="sbig", bufs=2))
        sexp_e = ctx.enter_context(tc.tile_pool(name="sexp_e", bufs=2))
        sexp_p = ctx.enter_context(tc.tile_pool(name="sexp_p", bufs=3))
        srow = ctx.enter_context(tc.tile_pool(name="srow", bufs=6))
        scols = ctx.enter_context(tc.tile_pool(name="scols", bufs=3))
        shd = ctx.enter_context(tc.tile_pool(name="shd", bufs=2))
        pz = ctx.enter_context(tc.tile_pool(name="pz", bufs=2, space="PSUM"))
        pou = ctx.enter_context(tc.tile_pool(name="pou", bufs=2, space="PSUM"))
        prow = ctx.enter_context(tc.tile_pool(name="prow", bufs=2, space="PSUM"))
        pmisc = ctx.enter_context(tc.tile_pool(name="pmisc", bufs=2, space="PSUM"))

        # ---------------- constants ----------------
        ident_f = const.tile([128, 128], F32)
        make_identity(nc, ident_f)
        ident_b = const.tile([128, 128], BF16)
        make_identity(nc, ident_b)
        ones_row_b = const.tile([1, N], BF16)
        nc.vector.memset(ones_row_b, 1.0)
        ones_row_f = const.tile([1, N], F32)
        nc.vector.memset(ones_row_f, 1.0)
        ones_sq_b = const.tile([128, 128], BF16)
        nc.vector.memset(ones_sq_b, 1.0)
        ones128_f = ones_row_f[:, 0:128]
        ones_col_b = const.tile([128, 1], BF16)
        nc.vector.memset(ones_col_b, 1.0)
        eps1 = const.tile([1, 1], F32)
        nc.vector.memset(eps1, LN_EPS)

        Wp_sb = const.tile([DIN, F], F32)
        nc.sync.dma_start(Wp_sb, Wp_d)
        Wp_bf = const.tile([DIN, F], BF16)
        nc.vector.tensor_copy(Wp_bf, Wp_sb)
        bp_col = const.tile([F, 1], F32)
        nc.sync.dma_start(bp_col, bp_d.rearrange("(f one) -> f one", one=1))

        # ---------------- weight loads + prep ----------------
        Wh_f = [stmp.tile([F, H, F], F32, tag="whf", name=f"WhF{l}")
                for l in range(L)]
        nc.sync.dma_start(Wh_f[0], Wh_d[0].rearrange("h i o -> i h o"))
        nc.gpsimd.dma_start(Wh_f[1], Wh_d[1].rearrange("h i o -> i h o"))
        Wh_b = [const.tile([F, H, F], BF16, name=f"WhB{l}") for l in range(L)]
        for l in range(L):
            nc.vector.tensor_copy(Wh_b[l], Wh_f[l])

        ah_all = const.tile([F, L * H, 2], F32)
        nc.sync.dma_start(ah_all, ah_d.rearrange("l h (t f) -> f (l h) t", t=2))
        ah_b = const.tile([F, L * H, 2], BF16)
        nc.vector.tensor_copy(ah_b, ah_all)
        ao_all = const.tile([F, L, 2], F32)
        nc.sync.dma_start(ao_all, ao_d.rearrange("l (t f) -> f l t", t=2))
        ao_b = const.tile([F, L, 2], BF16)
        nc.vector.tensor_copy(ao_b, ao_all)

        Wo_f = [stmp.tile([128, H, F], F32, tag="wof", name=f"WoF{l}")
                for l in range(L)]
        Wo_b = [const.tile([128, H, F], BF16, name=f"WoB{l}") for l in range(L)]
        for l in range(L):
            nc.gpsimd.dma_start(Wo_f[l], Wo_d[l].rearrange("(c p) f -> p c f", p=128))
            nc.vector.tensor_copy(Wo_b[l], Wo_f[l])

        g_row_f = [const.tile([1, F], F32, name=f"grf{l}") for l in range(L)]
        b_row_f = [const.tile([1, F], F32, name=f"brf{l}") for l in range(L)]
        g_col = [const.tile([F, 1], F32, name=f"gcol_{l}") for l in range(L)]
        for l in range(L):
            nc.sync.dma_start(g_row_f[l], g_d[l].rearrange("(one f) -> one f", one=1))
            nc.sync.dma_start(b_row_f[l], b_d[l].rearrange("(one f) -> one f", one=1))
            nc.sync.dma_start(g_col[l], g_d[l].rearrange("(f one) -> f one", one=1))

        # A[l][:, 2h+t] = Wh[l,h] @ a_heads[l,h,t]: via Wh^T transposes and
        # 2-col matmuls. Then rank-1 z stationaries M[l][h] = A1 (x) ones.
        A_bf = [const.tile([F, 2 * H], BF16, name=f"Abf{l}") for l in range(L)]
        A_f = [const.tile([F, 2 * H], F32, name=f"Af{l}") for l in range(L)]
        for l in range(L):
            A_ps = prow.tile([F, 2 * H], F32, tag="prow")
            for h in range(H):
                pt = pmisc.tile([128, 128], BF16, tag="pbig")
                nc.tensor.transpose(pt, Wh_b[l][:, h, :], ident_b)
                whT = stmp.tile([128, 128], BF16, tag="whT")
                nc.scalar.activation(whT, pt, AF.Copy)
                nc.tensor.matmul(A_ps[:, 2 * h:2 * h + 2], whT,
                                 ah_b[:, l * H + h, :], start=True, stop=True)
            nc.vector.tensor_copy(A_bf[l], A_ps)
            nc.vector.tensor_copy(A_f[l], A_ps)
        Mz = [[const.tile([F, 128], BF16, name=f"Mz{l}_{h}") for h in range(H)]
              for l in range(L)]
        for l in range(L):
            for h in range(H):
                nc.vector.tensor_scalar_mul(Mz[l][h], ones_sq_b,
                                            A_f[l][:, 2 * h:2 * h + 1])
        Mo = [const.tile([F, 128], BF16, name=f"Mo{l}") for l in range(L)]
        for l in range(L):
            nc.vector.tensor_scalar_mul(Mo[l], ones_sq_b, ao_all[:, l, 0:1])

        # ---------------- x -> xT -> input projection ----------------
        x_chunks = []
        for c in range(NCHUNK):
            xc = shd.tile([128, DIN], F32, tag="xchunk")
            nc.sync.dma_start(xc, x_d[bass.ts(c, 128), :])
            x_chunks.append(xc)
        xT_b = const.tile([DIN, N], BF16)
        ph = pmisc.tile([128, N], F32, tag="pbig")
        hT = sbig.tile([128, N], F32, tag="hT")
        hT_bf = sbig.tile([128, N], BF16, tag="hTb")
        for c in range(NCHUNK):
            pt = pmisc.tile([DIN, 128], F32, tag="pbig")
            nc.tensor.transpose(pt, x_chunks[c], ident_f)
            nc.vector.tensor_copy(xT_b[:, bass.ts(c, 128)], pt)
        nc.tensor.matmul(ph, Wp_bf, xT_b, start=True, stop=True)
        nc.scalar.activation(hT, ph, AF.Relu, bias=bp_col)
        nc.vector.tensor_copy(hT_bf, hT)

        # ---------------- adj -> maskT (bf16, transposed) ----------------
        maskT = const.tile([128, NCHUNK, N], BF16)
        for r in range(NCHUNK):
            ai = shd.tile([128, N], mybir.dt.int32, tag="adji")
            eng = nc.scalar if r % 2 == 0 else nc.sync
            eng.dma_start(ai, adj_d[bass.ts(r, 128), :])
            ab = shd.tile([128, N], BF16, tag="adjb")
            nc.vector.tensor_copy(ab, ai)
            for c in range(NCHUNK):
                pm = pmisc.tile([128, 128], BF16, tag="pbig")
                nc.tensor.transpose(pm, ab[:, bass.ts(c, 128)], ident_b)
                nc.scalar.activation(maskT[:, c, bass.ts(r, 128)], pm, AF.Copy)

        # ---------------- attention body helper ----------------
        def attn_body(M_stat, inT_b, s2col_of, projN_bf, den_ps, pou_t):
            """M_stat: [F,128] bf16 rank-1 z stationary; inT_b: [F,N] bf16;
            s2col_of(c) -> [128,1] f32 bias col; projN_bf: [128,N] bf16
            (n-part, f); den_ps: [1,N] psum; pou_t: [128,N] psum."""
            z_ps = pz.tile([128, N], F32, tag="z")
            nc.tensor.matmul(z_ps, M_stat, inT_b, start=True, stop=True)
            e_all = sexp_e.tile([128, NCHUNK, N], BF16, tag="e_all")
            for c in range(NCHUNK):
                nc.scalar.activation(e_all[:, c, :], z_ps, AF.Prelu,
                                     bias=s2col_of(c), alpha=ALPHA)
            p_all = sexp_p.tile([128, NCHUNK, N], BF16, tag="p_all")
            nc.scalar.activation(p_all, e_all, AF.Exp)
            nc.vector.tensor_tensor(p_all, p_all, maskT, OP.mult)
            for c in range(NCHUNK):
                nc.tensor.matmul(den_ps, ones_col_b, p_all[:, c, :],
                                 start=(c == 0), stop=(c == NCHUNK - 1))
            for c in range(NCHUNK):
                nc.tensor.matmul(pou_t, projN_bf[:, bass.ts(c, 128)],
                                 p_all[:, c, :],
                                 start=(c == 0), stop=(c == NCHUNK - 1))

        def norm_out(den_ps, pou_t, out_sb):
            """out_sb = pou_t * (1/den) broadcast."""
            r_f = srow.tile([1, N], F32, tag="row")
            nc.vector.reciprocal_approx_fast(r_f, den_ps)
            rep_ps = pmisc.tile([128, N], F32, tag="pbig")
            nc.tensor.matmul(rep_ps, ones128_f, r_f, start=True, stop=True)
            rep_sb = shd.tile([128, N], BF16, tag="rep_sb")
            nc.vector.tensor_copy(rep_sb, rep_ps)
            nc.vector.tensor_tensor(out_sb, pou_t, rep_sb, OP.mult)

        # ---------------- layers ----------------
        for l in range(L):
            residT = hT
            # ---- stage A: s2 columns for all heads + N-layout projections ----
            S12N_ps = prow.tile([128, NCHUNK, 2 * H], F32, tag="prow")
            for c in range(NCHUNK):
                nc.tensor.matmul(S12N_ps[:, c, :], hT_bf[:, bass.ts(c, 128)],
                                 A_bf[l], start=True, stop=True)
            s2a = scols.tile([128, NCHUNK, 2 * H], F32, tag="s2a")
            nc.vector.tensor_copy(s2a, S12N_ps)
            hprojN = []
            for h in range(H):
                pN = pmisc.tile([128, N], F32, tag="pbig")
                for c in range(NCHUNK):
                    nc.tensor.matmul(pN[:, bass.ts(c, 128)],
                                     hT_bf[:, bass.ts(c, 128)], Wh_b[l][:, h, :],
                                     start=True, stop=True)
                pn_bf = sproj.tile([128, N], BF16, tag="hprojN")
                nc.scalar.activation(pn_bf, pN, AF.Copy)
                hprojN.append(pn_bf)

            # ---- stage B: attention bodies per head ----
            multiT = []
            for h in range(H):
                den_ps = prow.tile([1, N], F32, tag="prow")
                pou_t = pou.tile([128, N], F32, tag="oU")
                attn_body(Mz[l][h], hT_bf,
                          lambda c, h=h: s2a[:, c, 2 * h + 1:2 * h + 2],
                          hprojN[h], den_ps, pou_t)
                outT = sbig.tile([128, N], BF16, tag="outT")
                norm_out(den_ps, pou_t, outT)
                ex = shd.tile([128, N], BF16, tag="elu_ex")
                nc.scalar.activation(ex, outT, AF.Exp)
                nc.vector.tensor_scalar(ex, ex, 1.0, -1.0, OP.min, OP.add)
                mh = smulti.tile([128, N], BF16, tag="multi")
                nc.vector.tensor_tensor(mh, outT, ex, OP.max)
                multiT.append(mh)

            # ---- out attention ----
            ph2 = pou.tile([128, N], F32, tag="oU")
            for h in range(H):
                nc.tensor.matmul(ph2, Wo_b[l][:, h, :], multiT[h],
                                 start=(h == 0), stop=(h == H - 1))
            h2_b = sbig.tile([128, N], BF16, tag="h2b")
            nc.scalar.activation(h2_b, ph2, AF.Copy)
            S2o_ps = prow.tile([128, NCHUNK, 2], F32, tag="prow")
            for c in range(NCHUNK):
                nc.tensor.matmul(S2o_ps[:, c, :], h2_b[:, bass.ts(c, 128)],
                                 ao_b[:, l, :], start=True, stop=True)
            s2o = scols.tile([128, NCHUNK, 2], F32, tag="s2o")
            nc.vector.tensor_copy(s2o, S2o_ps)
            pn2 = pmisc.tile([128, N], BF16, tag="pbig")
            for c in range(NCHUNK):
                nc.tensor.transpose(pn2[:, bass.ts(c, 128)],
                                    h2_b[:, bass.ts(c, 128)], ident_b)
            h2N_b = sproj.tile([128, N], BF16, tag="h2N")
            nc.scalar.activation(h2N_b, pn2, AF.Copy)

            deno = prow.tile([1, N], F32, tag="prow")
            pou_o = pou.tile([128, N], F32, tag="oU")
            attn_body(Mo[l], h2_b, lambda c: s2o[:, c, 1:2],
                      h2N_b, deno, pou_o)
            outsT = sbig.tile([128, N], F32, tag="outsT")
            norm_out(deno, pou_o, outsT)

            # ---- residual + LN over partition dim ----
            xs = sbig.tile([128, N], F32, tag="xs")
            nc.vector.tensor_tensor(xs, outsT, residT, OP.add)
            xs_b = sbig.tile([128, N], BF16, tag="xsb")
            nc.vector.tensor_copy(xs_b, xs)
            xsq_b = sbig.tile([128, N], BF16, tag="xsqb")
            nc.vector.tensor_tensor(xsq_b, xs_b, xs_b, OP.mult)
            pmu = prow.tile([1, N], F32, tag="prow")
            nc.tensor.matmul(pmu, ones_col_b, xs_b, start=True, stop=True)
            psq = prow.tile([1, N], F32, tag="prow")
            nc.tensor.matmul(psq, ones_col_b, xsq_b, start=True, stop=True)
            mu = srow.tile([1, N], F32, tag="row")
            nc.vector.tensor_scalar_mul(mu, pmu, 1.0 / F)
            mu2 = srow.tile([1, N], F32, tag="row")
            nc.vector.tensor_tensor(mu2, mu, mu, OP.mult)
            var = srow.tile([1, N], F32, tag="row")
            nc.vector.scalar_tensor_tensor(var, psq, 1.0 / F, mu2,
                                           OP.mult, OP.subtract)
            sd = srow.tile([1, N], F32, tag="row")
            nc.scalar.activation(sd, var, AF.Sqrt, bias=eps1)
            rstd = srow.tile([1, N], F32, tag="row")
            nc.vector.reciprocal_approx_fast(rstd, sd)
            r2 = srow.tile([1, N], F32, tag="row")
            nc.vector.scalar_tensor_tensor(r2, mu, -1.0, rstd, OP.mult, OP.mult)
            paff = pmisc.tile([128, N], F32, tag="pbig")
            nc.tensor.matmul(paff, g_row_f[l], r2, start=True, stop=False)
            nc.tensor.matmul(paff, b_row_f[l], ones_row_f, start=False, stop=True)
            rep_rstd = pmisc.tile([128, N], F32, tag="pbig")
            nc.tensor.matmul(rep_rstd, ones128_f, rstd, start=True, stop=True)
            y2 = sbig.tile([128, N], F32, tag="y2")
            nc.vector.scalar_tensor_tensor(y2, xs, g_col[l], rep_rstd,
                                           OP.mult, OP.mult)
            hT_new = sbig.tile([128, N], F32, tag="hT")
            nc.vector.tensor_tensor(hT_new, y2, paff, OP.add)
            if l < L - 1:
                hT = sbig.tile([128, N], F32, tag="hTr")
                nc.vector.tensor_scalar_max(hT, hT_new, 0.0)
                hT_bf = sbig.tile([128, N], BF16, tag="hTb")
                nc.scalar.activation(hT_bf, hT_new, AF.Relu)
            else:
                hT = hT_new

        # ---------------- output: transpose back ----------------
        for c in range(NCHUNK):
            po = pmisc.tile([128, 128], F32, tag="pbig")
            nc.tensor.transpose(po, hT[:, bass.ts(c, 128)], ident_f)
            osb = shd.tile([128, 128], F32, tag="osb")
            nc.scalar.activation(osb, po, AF.Copy)
            nc.sync.dma_start(out_d[bass.ts(c, 128), :], osb)

    nc.compile()
    return nc


def _get_nc():
    if "nc" not in _CACHE:
        _CACHE["nc"] = build_nc()
    return _CACHE["nc"]


def kernel(**inputs) -> np.ndarray:
    nc = _get_nc()
    shared = {k: np.ascontiguousarray(np.asarray(inputs[k], dtype=np.float32))
              for k in ("Wp", "bp", "W_heads", "a_heads", "W_out", "a_out",
                        "ln_g", "ln_b")}
    x = np.asarray(inputs["x"], dtype=np.float32)
    adj = np.asarray(inputs["adj"], dtype=np.int32)
    in_maps = [dict(x=np.ascontiguousarray(x[b]),
                    adj=np.ascontiguousarray(adj[b]), **shared)
               for b in range(B)]
    res = run_bass_kernel_spmd(nc, in_maps, core_ids=list(range(B)))
    return np.stack([res.results[b]["out"] for b in range(B)])


if __name__ == "__main__":
    rng = np.random.default_rng(0)
    inputs = dict(
        x=rng.normal(size=(B, N, DIN)).astype(np.float32),
        adj=rng.integers(0, 2, size=(B, N, N)).astype(np.int32),
        Wp=(rng.normal(size=(DIN, F)) * 0.12).astype(np.float32),
        bp=np.zeros(F, dtype=np.float32),
        W_heads=(rng.normal(size=(L, H, F, F)) * 0.08).astype(np.float32),
        a_heads=(rng.normal(size=(L, H, 2 * F)) * 0.08).astype(np.float32),
        W_out=(rng.normal(size=(L, H * F, F)) * 0.03).astype(np.float32),
        a_out=(rng.normal(size=(L, 2 * F)) * 0.08).astype(np.float32),
        ln_g=np.ones((L, F), dtype=np.float32),
        ln_b=np.zeros((L, F), dtype=np.float32),
    )
    out = kernel(**inputs)
    print("out", out.shape, out.dtype, np.abs(out).max())


# revision 12
# speedup vs baseline: 1.2213x; 1.0104x over previous
"""GAT spatio-temporal model Trainium2 kernel (v6).

Sharding: data-parallel over batch B=8 -> 8 NeuronCores (1 graph each).
Layout: feature-on-partition ("T" tensors are [F, N]); attention computed
in transposed [m, n] layout so softmax denominators come from ones-matmul
column sums and AV products hit PE directly.

v6 structure:
- z (s1 broadcast) from one bf16 matmul with prebuilt rank-1 stationary
  M_h = (Wh@a1) (x) ones; s2 bias columns from [128,16] chunk matmuls.
- softmax reciprocal broadcast via DRAM round-trip DMA (frees a PSUM
  bank -> AV pool gets 3 bufs for deeper head pipelining).
- input DMAs spread across all 4 queues, critical tensors first.
- stage A (projections) interleaved into stage B head loop (prefetch 2).
- Exp + mask split in halves to overlap scalar/DVE/PE per head.
- mask/weight-prep PSUM evacuations on DVE (scalar stays on softmax).

Shapes (hardcoded): B=8, N=512, Din=64, H=8, F=128, L=2.
"""
import os
import numpy as np
from contextlib import ExitStack

import concourse.bass as bass
import concourse.tile as tile
from concourse import bacc, mybir
from concourse.bass_utils import run_bass_kernel_spmd
from concourse.masks import make_identity

F32 = mybir.dt.float32
BF16 = mybir.dt.bfloat16
AF = mybir.ActivationFunctionType
OP = mybir.AluOpType

B, N, DIN, H, F, L = 8, 512, 64, 8, 128, 2
NCHUNK = N // 128  # 4
ALPHA = 0.2
LN_EPS = 1e-5

_CACHE = {}


def _bcast_row(ap_row):
    return bass.AP(tensor=ap_row.tensor, offset=ap_row.offset,
                   ap=[[0, 128], [1, N]])


def build_nc():
    nc = bacc.Bacc("TRN2", target_bir_lowering=False, debug=False)

    x_d = nc.dram_tensor("x", [N, DIN], F32, kind="ExternalInput").ap()
    adj_d = nc.dram_tensor("adj", [N, N], mybir.dt.int32, kind="ExternalInput").ap()
    Wp_d = nc.dram_tensor("Wp", [DIN, F], F32, kind="ExternalInput").ap()
    bp_d = nc.dram_tensor("bp", [F], F32, kind="ExternalInput").ap()
    Wh_d = nc.dram_tensor("W_heads", [L, H, F, F], F32, kind="ExternalInput").ap()
    ah_d = nc.dram_tensor("a_heads", [L, H, 2 * F], F32, kind="ExternalInput").ap()
    Wo_d = nc.dram_tensor("W_out", [L, H * F, F], F32, kind="ExternalInput").ap()
    ao_d = nc.dram_tensor("a_out", [L, 2 * F], F32, kind="ExternalInput").ap()
    g_d = nc.dram_tensor("ln_g", [L, F], F32, kind="ExternalInput").ap()
    b_d = nc.dram_tensor("ln_b", [L, F], F32, kind="ExternalInput").ap()
    out_d = nc.dram_tensor("out", [N, F], F32, kind="ExternalOutput").ap()
    scr_d = [nc.dram_tensor(f"scratch{i}", [1, N], F32, kind="ExternalOutput").ap()
             for i in range(4)]

    with tile.TileContext(nc) as tc, ExitStack() as ctx:
        const = ctx.enter_context(tc.tile_pool(name="const", bufs=1))
        stmp = ctx.enter_context(tc.tile_pool(name="stmp", bufs=2))
        sproj = ctx.enter_context(tc.tile_pool(name="sproj", bufs=5))
        smulti = ctx.enter_context(tc.tile_pool(name="smulti", bufs=9))
        sbig = ctx.enter_context(tc.tile_pool(name="sbig", bufs=2))
        sexp_e = ctx.enter_context(tc.tile_pool(name="sexp_e", bufs=2))
        sexp_p = ctx.enter_context(tc.tile_pool(name="sexp_p", bufs=3))
        srow = ctx.enter_context(tc.tile_pool(name="srow", bufs=6))
        srep = ctx.enter_context(tc.tile_pool(name="srep", bufs=3))
        scols = ctx.enter_context(tc.tile_pool(name="scols", bufs=3))
        shd = ctx.enter_context(tc.tile_pool(name="shd", bufs=2))
        pz = ctx.enter_context(tc.tile_pool(name="pz", bufs=2, space="PSUM"))
        pou = ctx.enter_context(tc.tile_pool(name="pou", bufs=3, space="PSUM"))
        prow = ctx.enter_context(tc.tile_pool(name="prow", bufs=2, space="PSUM"))
        pmisc = ctx.enter_context(tc.tile_pool(name="pmisc", bufs=1, space="PSUM"))

        # ---------------- input DMAs: critical first, spread queues --------
        x_chunks = []
        for c in range(NCHUNK):
            xc = shd.tile([128, DIN], F32, tag="xchunk", name=f"xc{c}")
            nc.sync.dma_start(xc, x_d[bass.ts(c, 128), :])
            x_chunks.append(xc)
        adj_i = []
        for r in range(NCHUNK):
            ai = shd.tile([128, N], mybir.dt.int32, tag="adji", name=f"ai{r}")
            eng = nc.scalar if r % 2 == 0 else nc.sync
            eng.dma_start(ai, adj_d[bass.ts(r, 128), :])
            adj_i.append(ai)
        Wh_f = [stmp.tile([F, H, F], F32, tag="whf", name=f"WhF{l}")
                for l in range(L)]
        Wo_f = [stmp.tile([128, H, F], F32, tag="wof", name=f"WoF{l}")
                for l in range(L)]
        nc.gpsimd.dma_start(Wh_f[0], Wh_d[0].rearrange("h i o -> i h o"))
        ah_all = const.tile([F, L * H, 2], F32)
        nc.gpsimd.dma_start(ah_all, ah_d.rearrange("l h (t f) -> f (l h) t", t=2))
        ao_all = const.tile([F, L, 2], F32)
        nc.gpsimd.dma_start(ao_all, ao_d.rearrange("l (t f) -> f l t", t=2))
        nc.gpsimd.dma_start(Wh_f[1], Wh_d[1].rearrange("h i o -> i h o"))
        nc.gpsimd.dma_start(Wo_f[0], Wo_d[0].rearrange("(c p) f -> p c f", p=128))
        nc.gpsimd.dma_start(Wo_f[1], Wo_d[1].rearrange("(c p) f -> p c f", p=128))

        Wp_sb = const.tile([DIN, F], F32)
        nc.sync.dma_start(Wp_sb, Wp_d)
        bp_col = const.tile([F, 1], F32)
        nc.sync.dma_start(bp_col, bp_d.rearrange("(f one) -> f one", one=1))
        g_row_f = [const.tile([1, F], F32, name=f"grf{l}") for l in range(L)]
        b_row_f = [const.tile([1, F], F32, name=f"brf{l}") for l in range(L)]
        g_col = [const.tile([F, 1], F32, name=f"gcol_{l}") for l in range(L)]
        for l in range(L):
            nc.scalar.dma_start(g_row_f[l], g_d[l].rearrange("(one f) -> one f", one=1))
            nc.scalar.dma_start(b_row_f[l], b_d[l].rearrange("(one f) -> one f", one=1))
            nc.scalar.dma_start(g_col[l], g_d[l].rearrange("(f one) -> f one", one=1))

        # ---------------- constants ----------------
        ident_f = const.tile([128, 128], F32)
        make_identity(nc, ident_f)
        ident_b = const.tile([128, 128], BF16)
        make_identity(nc, ident_b)
        ones_row_b = const.tile([1, N], BF16)
        nc.vector.memset(ones_row_b, 1.0)
        ones_row_f = const.tile([1, N], F32)
        nc.vector.memset(ones_row_f, 1.0)
        ones_sq_b = const.tile([128, 128], BF16)
        nc.vector.memset(ones_sq_b, 1.0)
        ones_col_b = const.tile([128, 1], BF16)
        nc.vector.memset(ones_col_b, 1.0)
        eps1 = const.tile([1, 1], F32)
        nc.vector.memset(eps1, LN_EPS)
        Wp_bf = const.tile([DIN, F], BF16)
        nc.vector.tensor_copy(Wp_bf, Wp_sb)

        # ---------------- weight prep ----------------
        Wh_b = [const.tile([F, H, F], BF16, name=f"WhB{l}") for l in range(L)]
        Wo_b = [const.tile([128, H, F], BF16, name=f"WoB{l}") for l in range(L)]
        ah_b = const.tile([F, L * H, 2], BF16)
        nc.vector.tensor_copy(ah_b, ah_all)
        ao_b = const.tile([F, L, 2], BF16)
        nc.vector.tensor_copy(ao_b, ao_all)
        for l in range(L):
            nc.vector.tensor_copy(Wh_b[l], Wh_f[l])
            nc.vector.tensor_copy(Wo_b[l], Wo_f[l])

        # A[l][:, 2h+t] = Wh[l,h] @ a_heads[l,h,t] via Wh^T transposes;
        # Mz[l][h] = A1 (x) ones (rank-1 z stationary).
        A_f = [const.tile([F, 2 * H], F32, name=f"Af{l}") for l in range(L)]
        A_bf = [const.tile([F, 2 * H], BF16, name=f"Abf{l}") for l in range(L)]
        for l in range(L):
            A_ps = prow.tile([F, 2 * H], F32, tag="prow")
            for h in range(H):
                pt = pmisc.tile([128, 128], BF16, tag="pbig", name=f"pt{l}_{h}")
                nc.tensor.transpose(pt, Wh_b[l][:, h, :], ident_b)
                whT = stmp.tile([128, 128], BF16, tag="whT", name=f"whT{l}_{h}")
                nc.vector.tensor_copy(whT, pt)
                nc.tensor.matmul(A_ps[:, 2 * h:2 * h + 2], whT,
                                 ah_b[:, l * H + h, :], start=True, stop=True)
            nc.vector.tensor_copy(A_bf[l], A_ps)
            nc.vector.tensor_copy(A_f[l], A_ps)
        Mz = [[const.tile([F, 128], BF16, name=f"Mz{l}_{h}") for h in range(H)]
              for l in range(L)]
        Mo = [const.tile([F, 128], BF16, name=f"Mo{l}") for l in range(L)]
        for l in range(L):
            for h in range(H):
                nc.vector.tensor_scalar_mul(Mz[l][h], ones_sq_b,
                                            A_f[l][:, 2 * h:2 * h + 1])
            nc.vector.tensor_scalar_mul(Mo[l], ones_sq_b, ao_all[:, l, 0:1])

        # ---------------- x -> xT -> input projection ----------------
        xT_b = const.tile([DIN, N], BF16)
        for c in range(NCHUNK):
            pt = pmisc.tile([DIN, 128], F32, tag="pbig", name=f"ptx{c}")
            nc.tensor.transpose(pt, x_chunks[c], ident_f)
            nc.vector.tensor_copy(xT_b[:, bass.ts(c, 128)], pt)
        ph = pz.tile([128, N], F32, tag="z")
        hT = sbig.tile([128, N], F32, tag="hT")
        hT_bf = sbig.tile([128, N], BF16, tag="hTb")
        nc.tensor.matmul(ph, Wp_bf, xT_b, start=True, stop=True)
        nc.scalar.activation(hT, ph, AF.Relu, bias=bp_col)
        nc.vector.tensor_copy(hT_bf, hT)

        # ---------------- adj -> maskT (bf16, transposed) ----------------
        maskT = const.tile([128, NCHUNK, N], BF16)
        for r in range(NCHUNK):
            ab = shd.tile([128, N], BF16, tag="adjb", name=f"ab{r}")
            nc.vector.tensor_copy(ab, adj_i[r])
            for c in range(NCHUNK):
                pm = pmisc.tile([128, 128], BF16, tag="pbig", name=f"pm{r}_{c}")
                nc.tensor.transpose(pm, ab[:, bass.ts(c, 128)], ident_b)
                nc.vector.tensor_copy(maskT[:, c, bass.ts(r, 128)], pm)

        # ---------------- helpers ----------------
        def stage_a_head(l, h):
            """Compute hprojN (n-part layout) for head h of layer l."""
            pN = pou.tile([128, N], F32, tag="oU", name=f"pN{l}_{h}")
            for c in range(NCHUNK):
                nc.tensor.matmul(pN[:, bass.ts(c, 128)],
                                 hT_bf[:, bass.ts(c, 128)], Wh_b[l][:, h, :],
                                 start=True, stop=True)
            pn_bf = sproj.tile([128, N], BF16, tag="hprojN", name=f"pn{l}_{h}")
            nc.vector.tensor_copy(pn_bf, pN)
            return pn_bf

        def attn_body(M_stat, inT_b, s2col_of, projN_bf, den_ps, pou_t):
            z_ps = pz.tile([128, N], F32, tag="z")
            nc.tensor.matmul(z_ps, M_stat, inT_b, start=True, stop=True)
            e_all = sexp_e.tile([128, NCHUNK, N], BF16, tag="e_all")
            for c in range(NCHUNK):
                nc.scalar.activation(e_all[:, c, :], z_ps, AF.Prelu,
                                     bias=s2col_of(c), alpha=ALPHA)
            p_all = sexp_p.tile([128, NCHUNK, N], BF16, tag="p_all")
            for half in range(2):
                sl = slice(2 * half, 2 * half + 2)
                nc.scalar.activation(p_all[:, sl, :], e_all[:, sl, :], AF.Exp)
                nc.vector.tensor_tensor(p_all[:, sl, :], p_all[:, sl, :],
                                        maskT[:, sl, :], OP.mult)
                for c in (2 * half, 2 * half + 1):
                    nc.tensor.matmul(den_ps, ones_col_b, p_all[:, c, :],
                                     start=(c == 0), stop=(c == NCHUNK - 1))
                for c in (2 * half, 2 * half + 1):
                    nc.tensor.matmul(pou_t, projN_bf[:, bass.ts(c, 128)],
                                     p_all[:, c, :],
                                     start=(c == 0), stop=(c == NCHUNK - 1))

        scr_idx = [0]

        def norm_out(den_ps, pou_t, out_sb, q):
            i = scr_idx[0] % 4
            scr_idx[0] += 1
            r_f = srow.tile([1, N], F32, tag="row")
            nc.vector.reciprocal_approx_fast(r_f, den_ps)
            q.dma_start(scr_d[i], r_f)
            rep_sb = srep.tile([128, N], F32, tag="rep")
            q.dma_start(rep_sb, _bcast_row(scr_d[i][0, :]))
            nc.vector.tensor_tensor(out_sb, pou_t, rep_sb, OP.mult)

        # ---------------- layers ----------------
        for l in range(L):
            residT = hT
            # stage A prologue: s2 columns for all heads + first projections
            S12N_ps = prow.tile([128, NCHUNK, 2 * H], F32, tag="prow")
            for c in range(NCHUNK):
                nc.tensor.matmul(S12N_ps[:, c, :], hT_bf[:, bass.ts(c, 128)],
                                 A_bf[l], start=True, stop=True)
            s2a = scols.tile([128, NCHUNK, 2 * H], F32, tag="s2a")
            nc.vector.tensor_copy(s2a, S12N_ps)
            hprojN = [stage_a_head(l, 0), stage_a_head(l, 1)]

            multiT = []
            for h in range(H):
                den_ps = prow.tile([1, N], F32, tag="prow")
                pou_t = pou.tile([128, N], F32, tag="oU")
                attn_body(Mz[l][h], hT_bf,
                          lambda c, h=h: s2a[:, c, 2 * h + 1:2 * h + 2],
                          hprojN[h], den_ps, pou_t)
                if h + 2 < H:
                    hprojN.append(stage_a_head(l, h + 2))
                outT = sbig.tile([128, N], BF16, tag="outT")
                norm_out(den_ps, pou_t, outT,
                         nc.sync if h % 2 == 0 else nc.gpsimd)
                ex = shd.tile([128, N], BF16, tag="elu_ex")
                nc.scalar.activation(ex, outT, AF.Exp)
                nc.vector.tensor_scalar(ex, ex, 1.0, -1.0, OP.min, OP.add)
                mh = smulti.tile([128, N], BF16, tag="multi")
                nc.vector.tensor_tensor(mh, outT, ex, OP.max)
                multiT.append(mh)

            # ---- out attention ----
            ph2 = pou.tile([128, N], F32, tag="oU")
            for h in range(H):
                nc.tensor.matmul(ph2, Wo_b[l][:, h, :], multiT[h],
                                 start=(h == 0), stop=(h == H - 1))
            h2_b = sbig.tile([128, N], BF16, tag="h2b")
            nc.scalar.activation(h2_b, ph2, AF.Copy)
            S2o_ps = prow.tile([128, NCHUNK, 2], F32, tag="prow")
            for c in range(NCHUNK):
                nc.tensor.matmul(S2o_ps[:, c, :], h2_b[:, bass.ts(c, 128)],
                                 ao_b[:, l, :], start=True, stop=True)
            s2o = scols.tile([128, NCHUNK, 2], F32, tag="s2o")
            nc.vector.tensor_copy(s2o, S2o_ps)
            pn2 = pmisc.tile([128, N], BF16, tag="pbig", name=f"pn2_{l}")
            for c in range(NCHUNK):
                nc.tensor.transpose(pn2[:, bass.ts(c, 128)],
                                    h2_b[:, bass.ts(c, 128)], ident_b)
            h2N_b = sproj.tile([128, N], BF16, tag="h2N")
            nc.vector.tensor_copy(h2N_b, pn2)

            deno = prow.tile([1, N], F32, tag="prow")
            pou_o = pou.tile([128, N], F32, tag="oU")
            attn_body(Mo[l], h2_b, lambda c: s2o[:, c, 1:2],
                      h2N_b, deno, pou_o)
            outsT = sbig.tile([128, N], F32, tag="outsT")
            norm_out(deno, pou_o, outsT, nc.sync)

            # ---- residual + LN over partition dim ----
            xs = sbig.tile([128, N], F32, tag="xs")
            nc.vector.tensor_tensor(xs, outsT, residT, OP.add)
            xs_b = sbig.tile([128, N], BF16, tag="xsb")
            nc.vector.tensor_copy(xs_b, xs)
            xsq_b = sbig.tile([128, N], BF16, tag="xsqb")
            nc.vector.tensor_tensor(xsq_b, xs, xs, OP.mult)
            pmu = prow.tile([1, N], F32, tag="prow")
            nc.tensor.matmul(pmu, ones_col_b, xs_b, start=True, stop=True)
            psq = prow.tile([1, N], F32, tag="prow")
            nc.tensor.matmul(psq, ones_col_b, xsq_b, start=True, stop=True)
            mu = srow.tile([1, N], F32, tag="row")
            nc.vector.tensor_scalar_mul(mu, pmu, 1.0 / F)
            mu2 = srow.tile([1, N], F32, tag="row")
            nc.vector.tensor_tensor(mu2, mu, mu, OP.mult)
            var = srow.tile([1, N], F32, tag="row")
            nc.vector.scalar_tensor_tensor(var, psq, 1.0 / F, mu2,
                                           OP.mult, OP.subtract)
            sd = srow.tile([1, N], F32, tag="row")
            nc.scalar.activation(sd, var, AF.Sqrt, bias=eps1)
            rstd = srow.tile([1, N], F32, tag="row")
            nc.vector.reciprocal_approx_fast(rstd, sd)
            r2 = srow.tile([1, N], F32, tag="row")
            nc.vector.scalar_tensor_tensor(r2, mu, -1.0, rstd, OP.mult, OP.mult)
            paff = pmisc.tile([128, N], F32, tag="pbig", name=f"paff{l}")
            nc.tensor.matmul(paff, g_row_f[l], r2, start=True, stop=False)
            nc.tensor.matmul(paff, b_row_f[l], ones_row_f, start=False, stop=True)
            rep_rstd = pz.tile([128, N], F32, tag="z")
            nc.tensor.matmul(rep_rstd, ones_row_f[:, 0:128], rstd,
                             start=True, stop=True)
            y2 = sbig.tile([128, N], F32, tag="y2")
            nc.vector.scalar_tensor_tensor(y2, xs, g_col[l], rep_rstd,
                                           OP.mult, OP.mult)
            hT_new = sbig.tile([128, N], F32, tag="hT")
            nc.vector.tensor_tensor(hT_new, y2, paff, OP.add)
            if l < L - 1:
                hT = sbig.tile([128, N], F32, tag="hTr")
                nc.vector.tensor_scalar_max(hT, hT_new, 0.0)
                hT_bf = sbig.tile([128, N], BF16, tag="hTb")
                nc.scalar.activation(hT_bf, hT_new, AF.Relu)
            else:
                hT = hT_new

        # ---------------- output: transpose back ----------------
        for c in range(NCHUNK):
            po = pmisc.tile([128, 128], F32, tag="pbig", name=f"po{c}")
            nc.tensor.transpose(po, hT[:, bass.ts(c, 128)], ident_f)
            osb = shd.tile([128, 128], F32, tag="osb", name=f"osb{c}")
            nc.scalar.activation(osb, po, AF.Copy)
            nc.sync.dma_start(out_d[bass.ts(c, 128), :], osb)

    nc.compile()
    return nc


def _get_nc():
    if "nc" not in _CACHE:
        _CACHE["nc"] = build_nc()
    return _CACHE["nc"]


def kernel(**inputs) -> np.ndarray:
    nc = _get_nc()
    shared = {k: np.ascontiguousarray(np.asarray(inputs[k], dtype=np.float32))
              for k in ("Wp", "bp", "W_heads", "a_heads", "W_out", "a_out",
                        "ln_g", "ln_b")}
    x = np.asarray(inputs["x"], dtype=np.float32)
    adj = np.asarray(inputs["adj"], dtype=np.int32)
    in_maps = [dict(x=np.ascontiguousarray(x[b]),
                    adj=np.ascontiguousarray(adj[b]), **shared)
               for b in range(B)]
    res = run_bass_kernel_spmd(nc, in_maps, core_ids=list(range(B)))
    return np.stack([res.results[b]["out"] for b in range(B)])


if __name__ == "__main__":
    rng = np.random.default_rng(0)
    inputs = dict(
        x=rng.normal(size=(B, N, DIN)).astype(np.float32),
        adj=rng.integers(0, 2, size=(B, N, N)).astype(np.int32),
        Wp=(rng.normal(size=(DIN, F)) * 0.12).astype(np.float32),
        bp=np.zeros(F, dtype=np.float32),
        W_heads=(rng.normal(size=(L, H, F, F)) * 0.08).astype(np.float32),
        a_heads=(rng.normal(size=(L, H, 2 * F)) * 0.08).astype(np.float32),
        W_out=(rng.normal(size=(L, H * F, F)) * 0.03).astype(np.float32),
        a_out=(rng.normal(size=(L, 2 * F)) * 0.08).astype(np.float32),
        ln_g=np.ones((L, F), dtype=np.float32),
        ln_b=np.zeros((L, F), dtype=np.float32),
    )
    out = kernel(**inputs)
    print("out", out.shape, out.dtype, np.abs(out).max())
